# revision 1
# baseline (speedup 1.0000x reference)
"""Mixtral expert-capacity top-2 MLP (per-expert grouped GEMM SwiGLU) on 8 NeuronCores.

Expert parallel: core e computes, for expert e,
    out_e = (silu(X_e @ W1_e) * (X_e @ W3_e)) @ W2_e
with X_e = dispatch_input[e] reshaped to (B*C, H).

Kernel structure per core (all matmuls in float32r -> fp22 inputs, fp32 accumulate):
  phase 0: transpose X (T,H) -> XT (H,T) via PE-transpose, XT SBUF-resident.
  pass 1:  hidden[f, t] = silu(W1.T X.T) * (W3.T X.T), computed f-tile (128) at a
           time with PSUM accumulation over H; staged to a DRAM scratch.
  pass 2:  out[t, h] = hidden.T @ W2 with PSUM accumulation over F in blocks of
           KFB=8 f-tiles, partials accumulated in an SBUF fp32 accumulator
           (one H-half at a time), then DMA'd out.

Hardware constraint worked around here: a fused fp32/fp32r Matmult (LDWEIGHTS+MM
pair) can encode only ONE sync wait.  Every DMA-produced tile that PE consumes is
first "observed" by a throwaway PE transpose (write-only dummy PSUM bank), so real
matmuls only ever wait on the single PSUM-slot-release semaphore.
"""

import os

os.environ.setdefault("MYCRO_LOCAL_CACHE", "1")

import numpy as np

E, B, C, H, F = 8, 4, 512, 2048, 7168
P = 128

TRACE = bool(int(os.environ.get("BASS_KERNEL_TRACE", "0")))
LAST_RESULT = None
_built = {}


def _build(T, Hd, Fd):
    import concourse.bass as bass
    import concourse.mybir as mybir
    from concourse import bacc
    import concourse.tile as tile
    from concourse.masks import make_identity

    f32 = mybir.dt.float32
    f32r = mybir.dt.float32r
    Silu = mybir.ActivationFunctionType.Silu

    assert T % P == 0 and Hd % P == 0 and Fd % P == 0
    NT = T // P          # token tiles
    NH = Hd // P         # H contraction tiles
    NF = Fd // P         # F tiles
    TCH = min(512, T)    # pass-1 moving-dim chunk (tokens)
    NTC = T // TCH
    KFB = 8 if NF % 8 == 0 else 1   # pass-2 f-tiles per PSUM accumulation block
    NKFB = NF // KFB
    NHF = 2              # pass-2 H halves (SBUF accumulator covers T x Hd/NHF)
    assert Hd % NHF == 0
    HH = Hd // NHF
    NQ = min(512, HH)    # pass-2 moving-dim chunk (H)
    NNQ = HH // NQ

    nc = bacc.Bacc(None, target_bir_lowering=False)
    x = nc.declare_dram_parameter("x", [T, Hd], f32, isOutput=False)
    w1 = nc.declare_dram_parameter("w1", [Hd, Fd], f32, isOutput=False)
    w2 = nc.declare_dram_parameter("w2", [Fd, Hd], f32, isOutput=False)
    w3 = nc.declare_dram_parameter("w3", [Hd, Fd], f32, isOutput=False)
    out = nc.declare_dram_parameter("out", [T, Hd], f32, isOutput=True)
    hid_dram = nc.dram_tensor("hid", [Fd, T], f32r)

    w1r = w1.rearrange("(ho hi) f -> hi ho f", hi=P)   # [128, NH, Fd]
    w3r = w3.rearrange("(ho hi) f -> hi ho f", hi=P)

    with tile.TileContext(nc) as tc:
        with tc.tile_pool(name="const", bufs=1) as const_pool:
            ident = const_pool.tile([P, P], f32)
            make_identity(nc, ident)

            def observe(t2d):
                """Absorb a DMA-completion wait into PE's clock: a throwaway
                bf16 ldweights reading only this tile.  No output -> no WAW
                self-wait; the next fused fp32r matmul reloads real weights."""
                nc.tensor.ldweights(t2d[:, 0:64].bitcast(mybir.dt.bfloat16))

            with tc.tile_pool(name="xt", bufs=1) as xt_pool:
                xt = xt_pool.tile([P, NH, T], f32r)  # XT[hi, ho, t]

                # ---- phase 0: X -> XT via PE transpose ----
                with tc.tile_pool(name="xload", bufs=2) as xload_pool, \
                     tc.tile_pool(name="pst", bufs=2, space="PSUM") as pst_pool:
                    for tt in range(NT):
                        x_t = xload_pool.tile([P, Hd], f32, tag="xld")
                        nc.gpsimd.dma_start(out=x_t, in_=x[tt * P:(tt + 1) * P, :])
                        observe(x_t[:, 0:P])
                        for ho in range(NH):
                            pt = pst_pool.tile([P, P], f32, tag="pst")
                            nc.tensor.transpose(pt, x_t[:, ho * P:(ho + 1) * P], ident)
                            nc.vector.tensor_copy(
                                out=xt[:, ho, tt * P:(tt + 1) * P], in_=pt
                            )

                tc.strict_bb_all_engine_barrier()
                observe(ident)  # re-observe post-barrier on a dep-free tile

                # ---- pass 1: hidden = silu(X@W1) * (X@W3), layout [f, t] ----
                with tc.tile_pool(name="w1c", bufs=2) as w1_pool, \
                     tc.tile_pool(name="w3c", bufs=2) as w3_pool, \
                     tc.tile_pool(name="hidst", bufs=4) as hid_pool, \
                     tc.tile_pool(name="ps_mm", bufs=3, space="PSUM") as ps_pool:
                    for ft in range(NF):
                        w1c = w1_pool.tile([P, NH, P], f32r, tag="w1c")
                        nc.gpsimd.dma_start(
                            out=w1c, in_=w1r[:, :, ft * P:(ft + 1) * P].bitcast(f32r)
                        )
                        observe(w1c[:, 0, :])
                        w3c = w3_pool.tile([P, NH, P], f32r, tag="w3c")
                        nc.gpsimd.dma_start(
                            out=w3c, in_=w3r[:, :, ft * P:(ft + 1) * P].bitcast(f32r)
                        )
                        observe(w3c[:, 0, :])
                        for tch in range(NTC):
                            tsl = slice(tch * TCH, (tch + 1) * TCH)
                            ps1 = ps_pool.tile([P, TCH], f32, tag="ps1")
                            ps3 = ps_pool.tile([P, TCH], f32, tag="ps3")
                            for ho in range(NH):
                                nc.tensor.matmul(
                                    ps1,
                                    w1c[:, ho, :],
                                    xt[:, ho, tsl],
                                    start=(ho == 0), stop=(ho == NH - 1),
                                )
                            for ho in range(NH):
                                nc.tensor.matmul(
                                    ps3,
                                    w3c[:, ho, :],
                                    xt[:, ho, tsl],
                                    start=(ho == 0), stop=(ho == NH - 1),
                                )
                            # ps1 released by ACT only; ps3 by DVE only
                            sl = hid_pool.tile([P, TCH], f32, tag="silu")
                            nc.scalar.activation(out=sl, in_=ps1, func=Silu)
                            hb = hid_pool.tile([P, TCH], f32r, tag="hid")
                            nc.vector.tensor_mul(hb, sl, ps3)
                            nc.gpsimd.dma_start(
                                out=hid_dram[ft * P:(ft + 1) * P, tsl], in_=hb
                            )

            tc.strict_bb_all_engine_barrier()
            observe(ident)

            # ---- pass 2: out = hidden.T @ W2 ----
            with tc.tile_pool(name="oa", bufs=1) as oa_pool, \
                 tc.tile_pool(name="hld", bufs=KFB + 2) as hld_pool, \
                 tc.tile_pool(name="w2ld", bufs=KFB + 2) as w2_pool, \
                 tc.tile_pool(name="ps2", bufs=6, space="PSUM") as ps2_pool:
                for nh in range(NHF):
                    oa = oa_pool.tile([P, NT, HH], f32, tag="oa")
                    for kfb in range(NKFB):
                        hcs = []
                        wcs = []
                        for j in range(KFB):
                            kf = kfb * KFB + j
                            hc = hld_pool.tile([P, T], f32r, tag="hc")
                            nc.gpsimd.dma_start(
                                out=hc, in_=hid_dram[kf * P:(kf + 1) * P, :]
                            )
                            observe(hc[:, 0:P])
                            wc = w2_pool.tile([P, HH], f32r, tag="wc")
                            nc.gpsimd.dma_start(
                                out=wc,
                                in_=w2[
                                    kf * P:(kf + 1) * P, nh * HH:(nh + 1) * HH
                                ].bitcast(f32r),
                            )
                            observe(wc[:, 0:P])
                            hcs.append(hc)
                            wcs.append(wc)
                        for mt in range(NT):
                            for nq in range(NNQ):
                                ps = ps2_pool.tile([P, NQ], f32, tag="ps2")
                                for j in range(KFB):
                                    nc.tensor.matmul(
                                        ps,
                                        hcs[j][:, mt * P:(mt + 1) * P],
                                        wcs[j][:, nq * NQ:(nq + 1) * NQ],
                                        start=(j == 0), stop=(j == KFB - 1),
                                    )
                                osl = oa[:, mt, nq * NQ:(nq + 1) * NQ]
                                if kfb == 0:
                                    nc.vector.tensor_copy(out=osl, in_=ps)
                                else:
                                    nc.vector.tensor_add(osl, osl, ps)
                    for mt in range(NT):
                        nc.gpsimd.dma_start(
                            out=out[mt * P:(mt + 1) * P, nh * HH:(nh + 1) * HH],
                            in_=oa[:, mt, :],
                        )
    nc.finalize()
    return nc


def _get_nc(T, Hd, Fd):
    key = (T, Hd, Fd)
    if key not in _built:
        _built[key] = _build(T, Hd, Fd)
    return _built[key]


def _run(x, w1, w2, w3):
    """x: (E, T, H); w1/w3: (E, H, F); w2: (E, F, H). Returns (E, T, H)."""
    from concourse.bass_utils import run_bass_kernel_spmd

    global LAST_RESULT
    Ne, T, Hd = x.shape
    Fd = w1.shape[-1]
    nc = _get_nc(T, Hd, Fd)
    in_maps = [
        {
            "x": np.ascontiguousarray(x[e], dtype=np.float32),
            "w1": np.ascontiguousarray(w1[e], dtype=np.float32),
            "w2": np.ascontiguousarray(w2[e], dtype=np.float32),
            "w3": np.ascontiguousarray(w3[e], dtype=np.float32),
        }
        for e in range(Ne)
    ]
    br = run_bass_kernel_spmd(nc, in_maps, core_ids=list(range(Ne)), trace=TRACE)
    LAST_RESULT = br
    return np.stack([br.results[e]["out"] for e in range(Ne)], axis=0)


def kernel(dispatch_input, w1, w2, w3):
    Ne, Bb, Cc, Hd = dispatch_input.shape
    xs = np.ascontiguousarray(
        np.asarray(dispatch_input, dtype=np.float32).reshape(Ne, Bb * Cc, Hd)
    )
    o = _run(xs, np.asarray(w1), np.asarray(w2), np.asarray(w3))
    return np.ascontiguousarray(o.reshape(Ne, Bb, Cc, Hd)).astype(np.float32)



# revision 3
# speedup vs baseline: 1.1282x; 1.1282x over previous
"""Mixtral expert-capacity top-2 MLP (per-expert grouped GEMM SwiGLU) on 8 NeuronCores.

Expert parallel: core e computes, for expert e,
    out_e = (silu(X_e @ W1_e) * (X_e @ W3_e)) @ W2_e
with X_e = dispatch_input[e] reshaped to (B*C, H).

Kernel structure per core (pass-1 matmuls in float32r -> fp32 accumulate;
pass-2 matmuls in bf16 -> fp32 accumulate):
  phase 0: transpose X (T,H) -> XT (H,T) via PE-transpose, XT SBUF-resident.
           No barrier after: pass 1 starts as soon as its first token chunk
           of XT is ready (observe() absorbs the DVE-completion waits).
  pass 1:  hidden[f, t] = silu(W1.T X.T) * (W3.T X.T), computed f-tile (128)
           at a time with PSUM accumulation over H; the silu*mul writes a
           bf16 [128, T] row tile staged to a DRAM scratch in one DMA.
  pass 2:  out[t, h] = hidden.T @ W2 in bf16 with PSUM accumulation over F in
           blocks of KFB=8 f-tiles, partials accumulated in per-token-tile
           fp32 SBUF accumulators (one H-half at a time), then DMA'd out.
           Deep prefetch (2*KFB bufs) keeps PE fed across block boundaries.

Hardware constraint worked around here: a fused fp32/fp32r Matmult (LDWEIGHTS+MM
pair) can encode only ONE sync wait.  Every DMA/DVE-produced tile that a fp32r
matmul consumes is first "observed" by a throwaway PE ldweights, so real
matmuls only ever wait on the single PSUM-slot-release semaphore.  bf16
matmuls (pass 2) have separate LDWEIGHTS and need no observes.
"""

import os

os.environ.setdefault("MYCRO_LOCAL_CACHE", "1")

import numpy as np

E, B, C, H, F = 8, 4, 512, 2048, 7168
P = 128

TRACE = bool(int(os.environ.get("BASS_KERNEL_TRACE", "0")))
LAST_RESULT = None
_built = {}


def _build(T, Hd, Fd):
    import concourse.bass as bass
    import concourse.mybir as mybir
    from concourse import bacc
    import concourse.tile as tile
    from concourse.masks import make_identity

    f32 = mybir.dt.float32
    f32r = mybir.dt.float32r
    bf16 = mybir.dt.bfloat16
    Silu = mybir.ActivationFunctionType.Silu

    assert T % P == 0 and Hd % P == 0 and Fd % P == 0
    NT = T // P          # token tiles
    NH = Hd // P         # H contraction tiles
    NF = Fd // P         # F tiles
    TCH = min(512, T)    # pass-1 moving-dim chunk (tokens)
    NTC = T // TCH
    TTC = TCH // P       # token tiles per chunk
    KFB = 8 if NF % 8 == 0 else 1   # pass-2 f-tiles per PSUM accumulation block
    NKFB = NF // KFB
    NHF = 2              # pass-2 H halves (SBUF accumulators cover T x Hd/NHF)
    assert Hd % NHF == 0
    HH = Hd // NHF
    NQ = min(512, HH)    # pass-2 moving-dim chunk (H)
    NNQ = HH // NQ

    nc = bacc.Bacc(None, target_bir_lowering=False)
    x = nc.declare_dram_parameter("x", [T, Hd], f32, isOutput=False)
    w1 = nc.declare_dram_parameter("w1", [Hd, Fd], f32, isOutput=False)
    w2 = nc.declare_dram_parameter("w2", [Fd, Hd], f32, isOutput=False)
    w3 = nc.declare_dram_parameter("w3", [Hd, Fd], f32, isOutput=False)
    out = nc.declare_dram_parameter("out", [T, Hd], f32, isOutput=True)
    hid_dram = nc.dram_tensor("hid", [Fd, T], bf16)

    w1r = w1.rearrange("(ho hi) f -> hi ho f", hi=P)   # [128, NH, Fd]
    w3r = w3.rearrange("(ho hi) f -> hi ho f", hi=P)

    with tile.TileContext(nc) as tc:
        with tc.tile_pool(name="const", bufs=1) as const_pool:
            ident = const_pool.tile([P, P], f32)
            make_identity(nc, ident)

            def observe(t2d):
                """Absorb a DMA/DVE-completion wait into PE's clock: a
                throwaway bf16 ldweights reading only this tile.  No output ->
                no WAW self-wait; the next fused fp32r matmul reloads real
                weights."""
                nc.tensor.ldweights(t2d[:, 0:64].bitcast(mybir.dt.bfloat16))

            with tc.tile_pool(name="xt", bufs=1) as xt_pool:
                xt = xt_pool.tile([P, NH, T], f32r)  # XT[hi, ho, t]

                # ---- phase 0: X -> XT via PE transpose ----
                with tc.tile_pool(name="xload", bufs=3) as xload_pool, \
                     tc.tile_pool(name="pst", bufs=2, space="PSUM") as pst_pool:
                    for tt in range(NT):
                        x_t = xload_pool.tile([P, Hd], f32, tag="xld")
                        nc.gpsimd.dma_start(out=x_t, in_=x[tt * P:(tt + 1) * P, :])
                        observe(x_t[:, 0:P])
                        for ho in range(NH):
                            pt = pst_pool.tile([P, P], f32, tag="pst")
                            nc.tensor.transpose(pt, x_t[:, ho * P:(ho + 1) * P], ident)
                            nc.vector.tensor_copy(
                                out=xt[:, ho, tt * P:(tt + 1) * P], in_=pt
                            )

                # ---- pass 1: hidden = silu(X@W1) * (X@W3), layout [f, t] ----
                # No barrier: ft==0 observes absorb the XT-readiness waits.
                with tc.tile_pool(name="w1c", bufs=2) as w1_pool, \
                     tc.tile_pool(name="w3c", bufs=2) as w3_pool, \
                     tc.tile_pool(name="slp", bufs=4) as sl_pool, \
                     tc.tile_pool(name="hbp", bufs=3) as hb_pool, \
                     tc.tile_pool(name="ps_mm", bufs=4, space="PSUM") as ps_pool:
                    for ft in range(NF):
                        w1c = w1_pool.tile([P, NH, P], f32r, tag="w1c")
                        nc.gpsimd.dma_start(
                            out=w1c, in_=w1r[:, :, ft * P:(ft + 1) * P].bitcast(f32r)
                        )
                        observe(w1c[:, 0, :])
                        w3c = w3_pool.tile([P, NH, P], f32r, tag="w3c")
                        nc.gpsimd.dma_start(
                            out=w3c, in_=w3r[:, :, ft * P:(ft + 1) * P].bitcast(f32r)
                        )
                        observe(w3c[:, 0, :])
                        hb = hb_pool.tile([P, T], bf16, tag="hid")
                        for tch in range(NTC):
                            tsl = slice(tch * TCH, (tch + 1) * TCH)
                            if ft == 0:
                                # absorb phase-0 DVE-copy completion for this
                                # token chunk (once; PE's clock then covers it)
                                for ho in range(NH):
                                    observe(xt[:, ho, tsl])
                            ps1 = ps_pool.tile([P, TCH], f32, tag="ps1")
                            ps3 = ps_pool.tile([P, TCH], f32, tag="ps3")
                            for ho in range(NH):
                                nc.tensor.matmul(
                                    ps1,
                                    w1c[:, ho, :],
                                    xt[:, ho, tsl],
                                    start=(ho == 0), stop=(ho == NH - 1),
                                )
                            for ho in range(NH):
                                nc.tensor.matmul(
                                    ps3,
                                    w3c[:, ho, :],
                                    xt[:, ho, tsl],
                                    start=(ho == 0), stop=(ho == NH - 1),
                                )
                            # ps1 released by ACT only; ps3 by DVE only
                            sl = sl_pool.tile([P, TCH], f32, tag="silu")
                            nc.scalar.activation(out=sl, in_=ps1, func=Silu)
                            nc.vector.tensor_mul(hb[:, tsl], sl, ps3)
                        nc.gpsimd.dma_start(
                            out=hid_dram[ft * P:(ft + 1) * P, :], in_=hb
                        )

            tc.strict_bb_all_engine_barrier()
            observe(ident)

            # ---- pass 2: out = hidden.T @ W2 (bf16 x bf16 -> fp32) ----
            with tc.tile_pool(name="oa", bufs=NT + 8) as oa_pool, \
                 tc.tile_pool(name="hld", bufs=2 * KFB) as hld_pool, \
                 tc.tile_pool(name="w2ld", bufs=2 * KFB) as w2_pool, \
                 tc.tile_pool(name="ps2", bufs=6, space="PSUM") as ps2_pool:
                for nh in range(NHF):
                    oas = [
                        oa_pool.tile([P, HH], f32, tag="oa", name=f"oa{nh}_{mt}")
                        for mt in range(NT)
                    ]
                    for kfb in range(NKFB):
                        hcs = []
                        wcs = []
                        for j in range(KFB):
                            kf = kfb * KFB + j
                            hc = hld_pool.tile([P, T], bf16, tag="hc")
                            nc.gpsimd.dma_start(
                                out=hc, in_=hid_dram[kf * P:(kf + 1) * P, :]
                            )
                            wc = w2_pool.tile([P, HH], bf16, tag="wc")
                            nc.gpsimd.dma_start(
                                out=wc,
                                in_=w2[kf * P:(kf + 1) * P, nh * HH:(nh + 1) * HH],
                            )
                            hcs.append(hc)
                            wcs.append(wc)
                        for mt in range(NT):
                            for nq in range(NNQ):
                                ps = ps2_pool.tile([P, NQ], f32, tag="ps2")
                                for j in range(KFB):
                                    nc.tensor.matmul(
                                        ps,
                                        hcs[j][:, mt * P:(mt + 1) * P],
                                        wcs[j][:, nq * NQ:(nq + 1) * NQ],
                                        start=(j == 0), stop=(j == KFB - 1),
                                    )
                                osl = oas[mt][:, nq * NQ:(nq + 1) * NQ]
                                if kfb == 0:
                                    nc.vector.tensor_copy(out=osl, in_=ps)
                                else:
                                    nc.vector.tensor_add(osl, osl, ps)
                    for mt in range(NT):
                        nc.gpsimd.dma_start(
                            out=out[mt * P:(mt + 1) * P, nh * HH:(nh + 1) * HH],
                            in_=oas[mt],
                        )
    nc.finalize()
    return nc


def _get_nc(T, Hd, Fd):
    key = (T, Hd, Fd)
    if key not in _built:
        _built[key] = _build(T, Hd, Fd)
    return _built[key]


def _run(x, w1, w2, w3):
    """x: (E, T, H); w1/w3: (E, H, F); w2: (E, F, H). Returns (E, T, H)."""
    from concourse.bass_utils import run_bass_kernel_spmd

    global LAST_RESULT
    Ne, T, Hd = x.shape
    Fd = w1.shape[-1]
    nc = _get_nc(T, Hd, Fd)
    in_maps = [
        {
            "x": np.ascontiguousarray(x[e], dtype=np.float32),
            "w1": np.ascontiguousarray(w1[e], dtype=np.float32),
            "w2": np.ascontiguousarray(w2[e], dtype=np.float32),
            "w3": np.ascontiguousarray(w3[e], dtype=np.float32),
        }
        for e in range(Ne)
    ]
    br = run_bass_kernel_spmd(nc, in_maps, core_ids=list(range(Ne)), trace=TRACE)
    LAST_RESULT = br
    return np.stack([br.results[e]["out"] for e in range(Ne)], axis=0)


def kernel(dispatch_input, w1, w2, w3):
    Ne, Bb, Cc, Hd = dispatch_input.shape
    xs = np.ascontiguousarray(
        np.asarray(dispatch_input, dtype=np.float32).reshape(Ne, Bb * Cc, Hd)
    )
    o = _run(xs, np.asarray(w1), np.asarray(w2), np.asarray(w3))
    return np.ascontiguousarray(o.reshape(Ne, Bb, Cc, Hd)).astype(np.float32)


# revision 4
# speedup vs baseline: 1.1705x; 1.0375x over previous
"""Mixtral expert-capacity top-2 MLP (per-expert grouped GEMM SwiGLU) on 8 NeuronCores.

Expert parallel: core e computes, for expert e,
    out_e = (silu(X_e @ W1_e) * (X_e @ W3_e)) @ W2_e
with X_e = dispatch_input[e] reshaped to (B*C, H).

All GEMM inputs are bf16 (cast host-side so DRAM traffic is halved); all
matmuls accumulate in fp32 PSUM.  W1/W3 are pre-tiled host-side to
[128, NF, NH, 128] so each f-tile's weight load is one fully-contiguous
4KB-per-partition-row DMA.

Kernel structure per core:
  phase 0: X (T,H) -> XT (H,T) via HWDGE DMA-transpose (bf16), no PE work.
           Runs on the HW-DGE queue, parallel with the SWDGE weight loads.
  pass 1:  hidden[f, t] = silu(W1.T X.T) * (W3.T X.T), computed f-tile (128)
           at a time with PSUM accumulation over H; silu*mul writes a bf16
           [128, T] row tile staged to a DRAM scratch in one DMA.
  pass 2:  out[t, h] = hidden.T @ W2 with PSUM accumulation over F in blocks
           of KFB=8 f-tiles, partials accumulated in per-token-tile fp32 SBUF
           accumulators (one H-half at a time), then DMA'd out via HWDGE.
           Deep prefetch (2*KFB bufs) keeps PE fed across block boundaries.

No barriers: Tile tracks the XT RAW deps (SBUF) and the hidden-scratch RAW
deps (DRAM) and emits minimal waits; bf16 matmuls have separate LDWEIGHTS so
the fused-fp32r single-wait limitation does not apply.
"""

import os

os.environ.setdefault("MYCRO_LOCAL_CACHE", "1")

import numpy as np
import ml_dtypes

BF16 = ml_dtypes.bfloat16

E, B, C, H, F = 8, 4, 512, 2048, 7168
P = 128

TRACE = bool(int(os.environ.get("BASS_KERNEL_TRACE", "0")))
LAST_RESULT = None
_built = {}


def _build(T, Hd, Fd):
    import concourse.bass as bass
    import concourse.mybir as mybir
    from concourse import bacc
    import concourse.tile as tile

    f32 = mybir.dt.float32
    bf16 = mybir.dt.bfloat16
    Silu = mybir.ActivationFunctionType.Silu

    assert T % P == 0 and Hd % P == 0 and Fd % P == 0
    NT = T // P          # token tiles
    NH = Hd // P         # H contraction tiles
    NF = Fd // P         # F tiles
    TCH = min(512, T)    # pass-1 moving-dim chunk (tokens)
    NTC = T // TCH
    KFB = 8 if NF % 8 == 0 else 1   # pass-2 f-tiles per PSUM accumulation block
    NKFB = NF // KFB
    NHF = 2              # pass-2 H halves (SBUF accumulators cover T x Hd/NHF)
    assert Hd % NHF == 0
    HH = Hd // NHF
    NQ = min(512, HH)    # pass-2 moving-dim chunk (H)
    NNQ = HH // NQ
    XTC = min(1024, T)   # phase-0 DMA-transpose token block
    NXT = T // XTC

    nc = bacc.Bacc(None, target_bir_lowering=False)
    x = nc.declare_dram_parameter("x", [T, Hd], bf16, isOutput=False)
    w1 = nc.declare_dram_parameter("w1", [P, NF, NH, P], bf16, isOutput=False)
    w2 = nc.declare_dram_parameter("w2", [Fd, Hd], bf16, isOutput=False)
    w3 = nc.declare_dram_parameter("w3", [P, NF, NH, P], bf16, isOutput=False)
    out = nc.declare_dram_parameter("out", [T, Hd], f32, isOutput=True)
    hid_dram = nc.dram_tensor("hid", [Fd, T], bf16)

    with tile.TileContext(nc) as tc:
        with tc.tile_pool(name="xt", bufs=1) as xt_pool:
            xt = xt_pool.tile([P, NH, T], bf16)  # XT[hi, ho, t]

            # ---- phase 0: X -> XT via HWDGE DMA-transpose ----
            for tb in range(NXT):
                xsl = slice(tb * XTC, (tb + 1) * XTC)
                for ho in range(NH):
                    nc.sync.dma_start_transpose(
                        out=xt[:, ho, xsl], in_=x[xsl, ho * P:(ho + 1) * P]
                    )

            # ---- pass 1: hidden = silu(X@W1) * (X@W3), layout [f, t] ----
            with tc.tile_pool(name="w1c", bufs=3) as w1_pool, \
                 tc.tile_pool(name="w3c", bufs=3) as w3_pool, \
                 tc.tile_pool(name="slp", bufs=4) as sl_pool, \
                 tc.tile_pool(name="hbp", bufs=3) as hb_pool, \
                 tc.tile_pool(name="ps_mm", bufs=4, space="PSUM") as ps_pool:
                for ft in range(NF):
                    w1c = w1_pool.tile([P, NH, P], bf16, tag="w1c")
                    nc.gpsimd.dma_start(out=w1c, in_=w1[:, ft])
                    w3c = w3_pool.tile([P, NH, P], bf16, tag="w3c")
                    nc.gpsimd.dma_start(out=w3c, in_=w3[:, ft])
                    hb = hb_pool.tile([P, T], bf16, tag="hid")
                    for tch in range(NTC):
                        tsl = slice(tch * TCH, (tch + 1) * TCH)
                        ps1 = ps_pool.tile([P, TCH], f32, tag="ps1")
                        ps3 = ps_pool.tile([P, TCH], f32, tag="ps3")
                        for ho in range(NH):
                            nc.tensor.matmul(
                                ps1,
                                w1c[:, ho, :],
                                xt[:, ho, tsl],
                                start=(ho == 0), stop=(ho == NH - 1),
                            )
                        for ho in range(NH):
                            nc.tensor.matmul(
                                ps3,
                                w3c[:, ho, :],
                                xt[:, ho, tsl],
                                start=(ho == 0), stop=(ho == NH - 1),
                            )
                        # ps1 released by ACT only; ps3 by DVE only
                        sl = sl_pool.tile([P, TCH], f32, tag="silu")
                        nc.scalar.activation(out=sl, in_=ps1, func=Silu)
                        nc.vector.tensor_mul(hb[:, tsl], sl, ps3)
                    nc.gpsimd.dma_start(
                        out=hid_dram[ft * P:(ft + 1) * P, :], in_=hb
                    )

        # ---- pass 2: out = hidden.T @ W2 (bf16 x bf16 -> fp32) ----
        with tc.tile_pool(name="oa", bufs=NT + 8) as oa_pool, \
             tc.tile_pool(name="hld", bufs=2 * KFB) as hld_pool, \
             tc.tile_pool(name="w2ld", bufs=2 * KFB) as w2_pool, \
             tc.tile_pool(name="ps2", bufs=6, space="PSUM") as ps2_pool:
            for nh in range(NHF):
                oas = [
                    oa_pool.tile([P, HH], f32, tag="oa", name=f"oa{nh}_{mt}")
                    for mt in range(NT)
                ]
                for kfb in range(NKFB):
                    hcs = []
                    wcs = []
                    for j in range(KFB):
                        kf = kfb * KFB + j
                        hc = hld_pool.tile([P, T], bf16, tag="hc")
                        nc.gpsimd.dma_start(
                            out=hc, in_=hid_dram[kf * P:(kf + 1) * P, :]
                        )
                        wc = w2_pool.tile([P, HH], bf16, tag="wc")
                        nc.gpsimd.dma_start(
                            out=wc,
                            in_=w2[kf * P:(kf + 1) * P, nh * HH:(nh + 1) * HH],
                        )
                        hcs.append(hc)
                        wcs.append(wc)
                    for mt in range(NT):
                        for nq in range(NNQ):
                            ps = ps2_pool.tile([P, NQ], f32, tag="ps2")
                            for j in range(KFB):
                                nc.tensor.matmul(
                                    ps,
                                    hcs[j][:, mt * P:(mt + 1) * P],
                                    wcs[j][:, nq * NQ:(nq + 1) * NQ],
                                    start=(j == 0), stop=(j == KFB - 1),
                                )
                            osl = oas[mt][:, nq * NQ:(nq + 1) * NQ]
                            if kfb == 0:
                                nc.vector.tensor_copy(out=osl, in_=ps)
                            else:
                                nc.vector.tensor_add(osl, osl, ps)
                for mt in range(NT):
                    nc.sync.dma_start(
                        out=out[mt * P:(mt + 1) * P, nh * HH:(nh + 1) * HH],
                        in_=oas[mt],
                    )
    nc.finalize()
    return nc


def _get_nc(T, Hd, Fd):
    key = (T, Hd, Fd)
    if key not in _built:
        _built[key] = _build(T, Hd, Fd)
    return _built[key]


def _prep_w13(w, NH, NF):
    """[H, F] fp32 -> [128, NF, NH, 128] bf16, so w[:, ft] is the f-tile's
    weight block [hi, ho, fi] with 4KB-contiguous per-partition rows."""
    Hd, Fd = w.shape
    return np.ascontiguousarray(
        w.astype(BF16).reshape(NH, P, NF, P).transpose(1, 2, 0, 3)
    )


def _run(x, w1, w2, w3):
    """x: (E, T, H); w1/w3: (E, H, F); w2: (E, F, H). Returns (E, T, H)."""
    from concourse.bass_utils import run_bass_kernel_spmd

    global LAST_RESULT
    Ne, T, Hd = x.shape
    Fd = w1.shape[-1]
    NH, NF = Hd // P, Fd // P
    nc = _get_nc(T, Hd, Fd)
    in_maps = [
        {
            "x": np.ascontiguousarray(np.asarray(x[e], dtype=np.float32).astype(BF16)),
            "w1": _prep_w13(np.asarray(w1[e], dtype=np.float32), NH, NF),
            "w2": np.ascontiguousarray(np.asarray(w2[e], dtype=np.float32).astype(BF16)),
            "w3": _prep_w13(np.asarray(w3[e], dtype=np.float32), NH, NF),
        }
        for e in range(Ne)
    ]
    br = run_bass_kernel_spmd(nc, in_maps, core_ids=list(range(Ne)), trace=TRACE)
    LAST_RESULT = br
    return np.stack([br.results[e]["out"] for e in range(Ne)], axis=0)


def kernel(dispatch_input, w1, w2, w3):
    Ne, Bb, Cc, Hd = dispatch_input.shape
    xs = np.asarray(dispatch_input, dtype=np.float32).reshape(Ne, Bb * Cc, Hd)
    o = _run(xs, np.asarray(w1), np.asarray(w2), np.asarray(w3))
    return np.ascontiguousarray(o.reshape(Ne, Bb, Cc, Hd)).astype(np.float32)


# revision 6
# speedup vs baseline: 1.2284x; 1.0494x over previous
"""Mixtral expert-capacity top-2 MLP (per-expert grouped GEMM SwiGLU) on 8 NeuronCores.

Expert parallel: core e computes, for expert e,
    out_e = (silu(X_e @ W1_e) * (X_e @ W3_e)) @ W2_e
with X_e = dispatch_input[e] reshaped to (B*C, H).

All GEMM inputs are bf16 (cast host-side so DRAM traffic is halved); all
matmuls accumulate in fp32 PSUM.  W1/W3 are pre-tiled host-side to
[128, NF, NH, 128] so each f-tile's weight load is one fully-contiguous
4KB-per-partition-row DMA.

Kernel structure per core:
  phase 0: X (T,H) -> XT (H,T) via HWDGE DMA-transpose (bf16), no PE work.
           Runs on the HW-DGE queue, parallel with the SWDGE weight loads.
  pass 1:  hidden[f, t] = silu(W1.T X.T) * (W3.T X.T), computed f-tile (128)
           at a time with PSUM accumulation over H; silu*mul writes a bf16
           [128, T] row tile staged to a DRAM scratch in one DMA.
  pass 2:  out[t, h] = hidden.T @ W2 with PSUM accumulation over F in blocks
           of KFB=8 f-tiles, partials accumulated in per-token-tile fp32 SBUF
           accumulators (one H-half at a time), then DMA'd out via HWDGE.
           Deep prefetch (2*KFB bufs) keeps PE fed across block boundaries.

No barriers: Tile tracks the XT RAW deps (SBUF) and the hidden-scratch RAW
deps (DRAM) and emits minimal waits; bf16 matmuls have separate LDWEIGHTS so
the fused-fp32r single-wait limitation does not apply.
"""

import os

os.environ.setdefault("MYCRO_LOCAL_CACHE", "1")

import numpy as np
import ml_dtypes

BF16 = ml_dtypes.bfloat16

E, B, C, H, F = 8, 4, 512, 2048, 7168
P = 128

TRACE = bool(int(os.environ.get("BASS_KERNEL_TRACE", "0")))
LAST_RESULT = None
_built = {}


def _build(T, Hd, Fd):
    import concourse.bass as bass
    import concourse.mybir as mybir
    from concourse import bacc
    import concourse.tile as tile

    f32 = mybir.dt.float32
    bf16 = mybir.dt.bfloat16
    Silu = mybir.ActivationFunctionType.Silu

    assert T % P == 0 and Hd % P == 0 and Fd % P == 0
    NT = T // P          # token tiles
    NH = Hd // P         # H contraction tiles
    NF = Fd // P         # F tiles
    TCH = min(512, T)    # pass-1 moving-dim chunk (tokens)
    NTC = T // TCH
    KFB = 8 if NF % 8 == 0 else 1   # pass-2 f-tiles per PSUM accumulation block
    NKFB = NF // KFB
    NHF = 2              # pass-2 H halves (SBUF accumulators cover T x Hd/NHF)
    assert Hd % NHF == 0
    HH = Hd // NHF
    NQ = min(512, HH)    # pass-2 moving-dim chunk (H)
    NNQ = HH // NQ
    XTC = min(512, T)    # phase-0 XT-load token block
    NXT = T // XTC

    nc = bacc.Bacc(None, target_bir_lowering=False)
    x = nc.declare_dram_parameter("x", [Hd, T], bf16, isOutput=False)  # host-transposed X.T
    w1 = nc.declare_dram_parameter("w1", [P, NF, NH, P], bf16, isOutput=False)
    w2 = nc.declare_dram_parameter("w2", [Fd, Hd], bf16, isOutput=False)
    w3 = nc.declare_dram_parameter("w3", [P, NF, NH, P], bf16, isOutput=False)
    out = nc.declare_dram_parameter("out", [T, Hd], f32, isOutput=True)
    hid_dram = nc.dram_tensor("hid", [Fd, T], bf16)

    with tile.TileContext(nc) as tc:
        with tc.tile_pool(name="xt", bufs=1) as xt_pool:
            xt = xt_pool.tile([P, NH, T], bf16)  # XT[hi, ho, t]

            # ---- phase 0: XT plain loads (X transposed host-side) ----
            for tb in range(NXT):
                xsl = slice(tb * XTC, (tb + 1) * XTC)
                for ho in range(NH):
                    nc.sync.dma_start(
                        out=xt[:, ho, xsl], in_=x[ho * P:(ho + 1) * P, xsl]
                    )

            # ---- pass 1: hidden = silu(X@W1) * (X@W3), layout [f, t] ----
            with tc.tile_pool(name="w1c", bufs=3) as w1_pool, \
                 tc.tile_pool(name="w3c", bufs=3) as w3_pool, \
                 tc.tile_pool(name="slp", bufs=4) as sl_pool, \
                 tc.tile_pool(name="hbp", bufs=3) as hb_pool, \
                 tc.tile_pool(name="ps_mm", bufs=4, space="PSUM") as ps_pool:
                for ft in range(NF):
                    w1c = w1_pool.tile([P, NH, P], bf16, tag="w1c")
                    nc.gpsimd.dma_start(out=w1c, in_=w1[:, ft])
                    w3c = w3_pool.tile([P, NH, P], bf16, tag="w3c")
                    nc.gpsimd.dma_start(out=w3c, in_=w3[:, ft])
                    hb = hb_pool.tile([P, T], bf16, tag="hid")
                    for tch in range(NTC):
                        tsl = slice(tch * TCH, (tch + 1) * TCH)
                        ps1 = ps_pool.tile([P, TCH], f32, tag="ps1")
                        ps3 = ps_pool.tile([P, TCH], f32, tag="ps3")
                        for ho in range(NH):
                            nc.tensor.matmul(
                                ps1,
                                w1c[:, ho, :],
                                xt[:, ho, tsl],
                                start=(ho == 0), stop=(ho == NH - 1),
                            )
                        for ho in range(NH):
                            nc.tensor.matmul(
                                ps3,
                                w3c[:, ho, :],
                                xt[:, ho, tsl],
                                start=(ho == 0), stop=(ho == NH - 1),
                            )
                        # ps1 released by ACT only; ps3 by DVE only
                        sl = sl_pool.tile([P, TCH], f32, tag="silu")
                        nc.scalar.activation(out=sl, in_=ps1, func=Silu)
                        nc.vector.tensor_mul(hb[:, tsl], sl, ps3)
                    nc.gpsimd.dma_start(
                        out=hid_dram[ft * P:(ft + 1) * P, :], in_=hb
                    )

        # ---- pass 2: out = hidden.T @ W2 (bf16 x bf16 -> fp32) ----
        with tc.tile_pool(name="oa", bufs=NT + 8) as oa_pool, \
             tc.tile_pool(name="hld", bufs=2 * KFB) as hld_pool, \
             tc.tile_pool(name="w2ld", bufs=2 * KFB) as w2_pool, \
             tc.tile_pool(name="ps2", bufs=6, space="PSUM") as ps2_pool:
            for nh in range(NHF):
                oas = [
                    oa_pool.tile([P, HH], f32, tag="oa", name=f"oa{nh}_{mt}")
                    for mt in range(NT)
                ]
                for kfb in range(NKFB):
                    hcs = []
                    wcs = []
                    for j in range(KFB):
                        kf = kfb * KFB + j
                        hc = hld_pool.tile([P, T], bf16, tag="hc")
                        nc.gpsimd.dma_start(
                            out=hc, in_=hid_dram[kf * P:(kf + 1) * P, :]
                        )
                        wc = w2_pool.tile([P, HH], bf16, tag="wc")
                        nc.sync.dma_start(
                            out=wc,
                            in_=w2[kf * P:(kf + 1) * P, nh * HH:(nh + 1) * HH],
                        )
                        hcs.append(hc)
                        wcs.append(wc)
                    for mt in range(NT):
                        for nq in range(NNQ):
                            ps = ps2_pool.tile([P, NQ], f32, tag="ps2")
                            for j in range(KFB):
                                nc.tensor.matmul(
                                    ps,
                                    hcs[j][:, mt * P:(mt + 1) * P],
                                    wcs[j][:, nq * NQ:(nq + 1) * NQ],
                                    start=(j == 0), stop=(j == KFB - 1),
                                )
                            osl = oas[mt][:, nq * NQ:(nq + 1) * NQ]
                            if kfb == 0:
                                nc.vector.tensor_copy(out=osl, in_=ps)
                            else:
                                nc.vector.tensor_add(osl, osl, ps)
                for mt in range(NT):
                    nc.sync.dma_start(
                        out=out[mt * P:(mt + 1) * P, nh * HH:(nh + 1) * HH],
                        in_=oas[mt],
                    )
    nc.finalize()
    return nc


def _get_nc(T, Hd, Fd):
    key = (T, Hd, Fd)
    if key not in _built:
        _built[key] = _build(T, Hd, Fd)
    return _built[key]


def _prep_w13(w, NH, NF):
    """[H, F] fp32 -> [128, NF, NH, 128] bf16, so w[:, ft] is the f-tile's
    weight block [hi, ho, fi] with 4KB-contiguous per-partition rows."""
    Hd, Fd = w.shape
    return np.ascontiguousarray(
        w.astype(BF16).reshape(NH, P, NF, P).transpose(1, 2, 0, 3)
    )


def _run(x, w1, w2, w3):
    """x: (E, T, H); w1/w3: (E, H, F); w2: (E, F, H). Returns (E, T, H)."""
    from concourse.bass_utils import run_bass_kernel_spmd

    global LAST_RESULT
    Ne, T, Hd = x.shape
    Fd = w1.shape[-1]
    NH, NF = Hd // P, Fd // P
    nc = _get_nc(T, Hd, Fd)
    in_maps = [
        {
            "x": np.ascontiguousarray(
                np.asarray(x[e], dtype=np.float32).astype(BF16).T
            ),
            "w1": _prep_w13(np.asarray(w1[e], dtype=np.float32), NH, NF),
            "w2": np.ascontiguousarray(np.asarray(w2[e], dtype=np.float32).astype(BF16)),
            "w3": _prep_w13(np.asarray(w3[e], dtype=np.float32), NH, NF),
        }
        for e in range(Ne)
    ]
    br = run_bass_kernel_spmd(nc, in_maps, core_ids=list(range(Ne)), trace=TRACE)
    LAST_RESULT = br
    return np.stack([br.results[e]["out"] for e in range(Ne)], axis=0)


def kernel(dispatch_input, w1, w2, w3):
    Ne, Bb, Cc, Hd = dispatch_input.shape
    xs = np.asarray(dispatch_input, dtype=np.float32).reshape(Ne, Bb * Cc, Hd)
    o = _run(xs, np.asarray(w1), np.asarray(w2), np.asarray(w3))
    return np.ascontiguousarray(o.reshape(Ne, Bb, Cc, Hd)).astype(np.float32)


# revision 7
# speedup vs baseline: 1.2289x; 1.0004x over previous
"""Mixtral expert-capacity top-2 MLP (per-expert grouped GEMM SwiGLU) on 8 NeuronCores.

Expert parallel: core e computes, for expert e,
    out_e = (silu(X_e @ W1_e) * (X_e @ W3_e)) @ W2_e
with X_e = dispatch_input[e] reshaped to (B*C, H).

All GEMM inputs are bf16 (cast host-side so DRAM traffic is halved); all
matmuls accumulate in fp32 PSUM.  W1/W3 are pre-tiled host-side to
[128, NF, NH, 128] so each f-tile's weight load is one fully-contiguous
4KB-per-partition-row DMA.

Kernel structure per core:
  phase 0: X (T,H) -> XT (H,T) via HWDGE DMA-transpose (bf16), no PE work.
           Runs on the HW-DGE queue, parallel with the SWDGE weight loads.
  pass 1:  hidden[f, t] = silu(W1.T X.T) * (W3.T X.T), computed f-tile (128)
           at a time with PSUM accumulation over H; silu*mul writes a bf16
           [128, T] row tile staged to a DRAM scratch in one DMA.
  pass 2:  out[t, h] = hidden.T @ W2 with PSUM accumulation over F in blocks
           of KFB=8 f-tiles, partials accumulated in per-token-tile fp32 SBUF
           accumulators (one H-half at a time), then DMA'd out via HWDGE.
           Deep prefetch (2*KFB bufs) keeps PE fed across block boundaries.

No barriers: Tile tracks the XT RAW deps (SBUF) and the hidden-scratch RAW
deps (DRAM) and emits minimal waits; bf16 matmuls have separate LDWEIGHTS so
the fused-fp32r single-wait limitation does not apply.
"""

import os

os.environ.setdefault("MYCRO_LOCAL_CACHE", "1")

import numpy as np
import ml_dtypes

BF16 = ml_dtypes.bfloat16

E, B, C, H, F = 8, 4, 512, 2048, 7168
P = 128

TRACE = bool(int(os.environ.get("BASS_KERNEL_TRACE", "0")))
LAST_RESULT = None
_built = {}


def _build(T, Hd, Fd):
    import concourse.bass as bass
    import concourse.mybir as mybir
    from concourse import bacc
    import concourse.tile as tile

    f32 = mybir.dt.float32
    bf16 = mybir.dt.bfloat16
    Silu = mybir.ActivationFunctionType.Silu

    assert T % P == 0 and Hd % P == 0 and Fd % P == 0
    NT = T // P          # token tiles
    NH = Hd // P         # H contraction tiles
    NF = Fd // P         # F tiles
    TCH = min(512, T)    # pass-1 moving-dim chunk (tokens)
    NTC = T // TCH
    KFB = 8 if NF % 8 == 0 else 1   # pass-2 f-tiles per PSUM accumulation block
    NKFB = NF // KFB
    NHF = 2              # pass-2 H halves (SBUF accumulators cover T x Hd/NHF)
    assert Hd % NHF == 0
    HH = Hd // NHF
    NQ = min(512, HH)    # pass-2 moving-dim chunk (H)
    NNQ = HH // NQ
    XTC = min(512, T)    # phase-0 XT-load token block
    NXT = T // XTC

    nc = bacc.Bacc(None, target_bir_lowering=False)
    x = nc.declare_dram_parameter("x", [Hd, T], bf16, isOutput=False)  # host-transposed X.T
    w1 = nc.declare_dram_parameter("w1", [P, NF, NH, P], bf16, isOutput=False)
    w2 = nc.declare_dram_parameter("w2", [Fd, Hd], bf16, isOutput=False)
    w3 = nc.declare_dram_parameter("w3", [P, NF, NH, P], bf16, isOutput=False)
    out = nc.declare_dram_parameter("out", [T, Hd], f32, isOutput=True)
    hid_dram = nc.dram_tensor("hid", [Fd, T], bf16)

    with tile.TileContext(nc) as tc:
        with tc.tile_pool(name="xt", bufs=1) as xt_pool:
            xt = xt_pool.tile([P, NH, T], bf16)  # XT[hi, ho, t]

            # ---- phase 0: XT plain loads (X transposed host-side) ----
            for tb in range(NXT):
                xsl = slice(tb * XTC, (tb + 1) * XTC)
                for ho in range(NH):
                    eng = nc.sync if ho % 2 == 0 else nc.scalar
                    eng.dma_start(
                        out=xt[:, ho, xsl], in_=x[ho * P:(ho + 1) * P, xsl]
                    )

            # ---- pass 1: hidden = silu(X@W1) * (X@W3), layout [f, t] ----
            with tc.tile_pool(name="w1c", bufs=3) as w1_pool, \
                 tc.tile_pool(name="w3c", bufs=3) as w3_pool, \
                 tc.tile_pool(name="slp", bufs=4) as sl_pool, \
                 tc.tile_pool(name="hbp", bufs=3) as hb_pool, \
                 tc.tile_pool(name="ps_mm", bufs=4, space="PSUM") as ps_pool:
                for ft in range(NF):
                    w1c = w1_pool.tile([P, NH, P], bf16, tag="w1c")
                    nc.gpsimd.dma_start(out=w1c, in_=w1[:, ft])
                    w3c = w3_pool.tile([P, NH, P], bf16, tag="w3c")
                    nc.gpsimd.dma_start(out=w3c, in_=w3[:, ft])
                    hb = hb_pool.tile([P, T], bf16, tag="hid")
                    for tch in range(NTC):
                        tsl = slice(tch * TCH, (tch + 1) * TCH)
                        ps1 = ps_pool.tile([P, TCH], f32, tag="ps1")
                        ps3 = ps_pool.tile([P, TCH], f32, tag="ps3")
                        for ho in range(NH):
                            nc.tensor.matmul(
                                ps1,
                                w1c[:, ho, :],
                                xt[:, ho, tsl],
                                start=(ho == 0), stop=(ho == NH - 1),
                            )
                        for ho in range(NH):
                            nc.tensor.matmul(
                                ps3,
                                w3c[:, ho, :],
                                xt[:, ho, tsl],
                                start=(ho == 0), stop=(ho == NH - 1),
                            )
                        # ps1 released by ACT only; ps3 by DVE only
                        sl = sl_pool.tile([P, TCH], f32, tag="silu")
                        nc.scalar.activation(out=sl, in_=ps1, func=Silu)
                        nc.vector.tensor_mul(hb[:, tsl], sl, ps3)
                    nc.scalar.dma_start(
                        out=hid_dram[ft * P:(ft + 1) * P, :], in_=hb
                    )

        # ---- pass 2: out = hidden.T @ W2 (bf16 x bf16 -> fp32) ----
        with tc.tile_pool(name="oa", bufs=NT + 8) as oa_pool, \
             tc.tile_pool(name="hld", bufs=2 * KFB) as hld_pool, \
             tc.tile_pool(name="w2ld", bufs=2 * KFB) as w2_pool, \
             tc.tile_pool(name="ps2", bufs=6, space="PSUM") as ps2_pool:
            for nh in range(NHF):
                oas = [
                    oa_pool.tile([P, HH], f32, tag="oa", name=f"oa{nh}_{mt}")
                    for mt in range(NT)
                ]
                for kfb in range(NKFB):
                    hcs = []
                    wcs = []
                    for j in range(KFB):
                        kf = kfb * KFB + j
                        hc = hld_pool.tile([P, T], bf16, tag="hc")
                        nc.gpsimd.dma_start(
                            out=hc, in_=hid_dram[kf * P:(kf + 1) * P, :]
                        )
                        wc = w2_pool.tile([P, HH], bf16, tag="wc")
                        nc.sync.dma_start(
                            out=wc,
                            in_=w2[kf * P:(kf + 1) * P, nh * HH:(nh + 1) * HH],
                        )
                        hcs.append(hc)
                        wcs.append(wc)
                    for mt in range(NT):
                        for nq in range(NNQ):
                            ps = ps2_pool.tile([P, NQ], f32, tag="ps2")
                            for j in range(KFB):
                                nc.tensor.matmul(
                                    ps,
                                    hcs[j][:, mt * P:(mt + 1) * P],
                                    wcs[j][:, nq * NQ:(nq + 1) * NQ],
                                    start=(j == 0), stop=(j == KFB - 1),
                                )
                            osl = oas[mt][:, nq * NQ:(nq + 1) * NQ]
                            if kfb == 0:
                                nc.vector.tensor_copy(out=osl, in_=ps)
                            else:
                                nc.vector.tensor_add(osl, osl, ps)
                for mt in range(NT):
                    nc.sync.dma_start(
                        out=out[mt * P:(mt + 1) * P, nh * HH:(nh + 1) * HH],
                        in_=oas[mt],
                    )
    nc.finalize()
    return nc


def _get_nc(T, Hd, Fd):
    key = (T, Hd, Fd)
    if key not in _built:
        _built[key] = _build(T, Hd, Fd)
    return _built[key]


def _prep_w13(w, NH, NF):
    """[H, F] fp32 -> [128, NF, NH, 128] bf16, so w[:, ft] is the f-tile's
    weight block [hi, ho, fi] with 4KB-contiguous per-partition rows."""
    Hd, Fd = w.shape
    return np.ascontiguousarray(
        w.astype(BF16).reshape(NH, P, NF, P).transpose(1, 2, 0, 3)
    )


def _run(x, w1, w2, w3):
    """x: (E, T, H); w1/w3: (E, H, F); w2: (E, F, H). Returns (E, T, H)."""
    from concourse.bass_utils import run_bass_kernel_spmd

    global LAST_RESULT
    Ne, T, Hd = x.shape
    Fd = w1.shape[-1]
    NH, NF = Hd // P, Fd // P
    nc = _get_nc(T, Hd, Fd)
    in_maps = [
        {
            "x": np.ascontiguousarray(
                np.asarray(x[e], dtype=np.float32).astype(BF16).T
            ),
            "w1": _prep_w13(np.asarray(w1[e], dtype=np.float32), NH, NF),
            "w2": np.ascontiguousarray(np.asarray(w2[e], dtype=np.float32).astype(BF16)),
            "w3": _prep_w13(np.asarray(w3[e], dtype=np.float32), NH, NF),
        }
        for e in range(Ne)
    ]
    br = run_bass_kernel_spmd(nc, in_maps, core_ids=list(range(Ne)), trace=TRACE)
    LAST_RESULT = br
    return np.stack([br.results[e]["out"] for e in range(Ne)], axis=0)


def kernel(dispatch_input, w1, w2, w3):
    Ne, Bb, Cc, Hd = dispatch_input.shape
    xs = np.asarray(dispatch_input, dtype=np.float32).reshape(Ne, Bb * Cc, Hd)
    o = _run(xs, np.asarray(w1), np.asarray(w2), np.asarray(w3))
    return np.ascontiguousarray(o.reshape(Ne, Bb, Cc, Hd)).astype(np.float32)
